# revision 4
# baseline (speedup 1.0000x reference)
"""CRF negative-log-likelihood loss on 8 Trainium2 NeuronCores.

Data-parallel over the batch: each core handles 64 of the 512 sequences,
runs the forward algorithm (log-partition) plus the gold-score reduction,
and returns three small partial tensors that the host sums into the
scalar loss.

The host hands each core TWO copies of its emit shard: the natural
[B, T, L] layout (gold score) and a transposed [L, T, B] layout (scan),
so no on-device transposes are needed.

Forward algorithm layout: scan state p[j, b] = exp(alpha[b, j] - offset)
in [L=64 partitions, B free], split into NG independent batch groups that
interleave on PE/DVE to hide the per-step PE->DVE->PE latency.  One step:
    p' = (expT^T @ p) * exp(emit_t - C_SHIFT)        (PE matmul + DVE mult)
C_SHIFT absorbs the typical per-step growth of alpha; a column-sum
renormalization every RENORM steps absorbs the remainder into Macc.

Gold score: one-hot tiles of the targets (broadcast is_equal); emit part
multiplies em*oh on DVE and row-sums each sub-range on ACT (activation
accum_out) into one column of a partials tile, reduced once at the end;
transition part via PSUM-accumulated pair-count matmuls oh_t^T @ oh_{t+1}
dotted with `transition` at the end.
"""

import sys

if "/opt/trn_rl_repo" not in sys.path:
    sys.path.insert(0, "/opt/trn_rl_repo")

import numpy as np

B, T, L = 512, 1024, 64
NCORES = 8
BL = B // NCORES  # 64 sequences per core
NG = 1  # scan batch groups (interleaved chains)
BG = BL // NG
T_START, T_STOP = 1, 2
C_SHIFT = 5.2  # per-step log-space rescale: ~ln(L * E[e^N]^2)
TC = 64  # time steps per pipeline chunk
NCHUNK = T // TC
RENORM = 50
N_STEPS = T - 2  # emit factors used: t = 1 .. T-2  (1022 of them)
NSUB = 4  # emit-score sub-ops per chunk
NPART = NCHUNK * NSUB  # emit-score partial columns

_CACHE = {}
TRACE = False  # set True (e.g. from test.py) to capture an NTFF profile
LAST_EXEC_NS = None
LAST_RESULT = None


def _split_multi_waits(nc, mybir, max_waits=1):
    """The walrus build in this container only encodes one sync-wait per
    instruction.  First drop same-engine waits that in-order execution
    already satisfies (wait value <= increments this engine has issued
    earlier in program order), then hoist any remaining extra waits onto
    NoOps inserted just before."""
    n_split = 0
    for f in nc.m.functions:
        for bb in f.blocks:
            insts = list(bb.instructions)
            # cumulative per-semaphore increments issued by each engine
            inc_count = {}
            out = []
            changed = False
            for ins in insts:
                si = getattr(ins, "sync_info", None)
                waits = list(si.on_wait) if si is not None and si.on_wait else []
                # PE reorders LDWEIGHTS ahead of in-flight matmuls, so only
                # strict-FIFO engines get the in-order elision.
                if waits and str(ins.engine) != "EngineType.PE":
                    eng = str(ins.engine)
                    kept = []
                    for w in waits:
                        key = (eng, w.ant_name)
                        if (
                            w.wait_mode == "sem-ge-imm"
                            and inc_count.get(key, 0) >= (w.wait_value or 0)
                        ):
                            changed = True
                            continue  # satisfied by in-order execution
                        kept.append(w)
                    waits = kept
                    if len(waits) != len(si.on_wait):
                        si.on_wait = waits
                if len(waits) > max_waits:
                    keep = waits[len(waits) - max_waits :]
                    hoist = waits[: len(waits) - max_waits]
                    for i, w in enumerate(hoist):
                        nop = mybir.InstNoOp(
                            name=f"{ins.name}-hw{i}", ins=[], outs=[]
                        )
                        nop.engine = ins.engine
                        nop.sync_info = mybir.SyncInfo(on_wait=[w], on_update=[])
                        out.append(nop)
                    si.on_wait = keep
                    changed = True
                    n_split += 1
                out.append(ins)
                if si is not None and si.on_update:
                    eng = str(ins.engine)
                    for u in si.on_update:
                        if getattr(u, "update_mode", None) == "sem-inc":
                            key = (eng, u.ant_name)
                            inc_count[key] = inc_count.get(key, 0) + (
                                u.update_value or 0
                            )
            if changed:
                bb.instructions = out
    return n_split


def _build(nreps=1):
    import concourse.bass as bass
    import concourse.mybir as mybir
    import concourse.tile as tile

    fp32 = mybir.dt.float32
    bf16 = mybir.dt.bfloat16
    i32 = mybir.dt.int32
    AOP = mybir.AluOpType
    AF = mybir.ActivationFunctionType
    AX = mybir.AxisListType

    nc = bass.Bass()
    emitT_d = nc.dram_tensor("emitT", [L, T, BL], fp32, kind="ExternalInput")
    emit_d = nc.dram_tensor("emit", [BL, T, L], fp32, kind="ExternalInput")
    tgt_d = nc.dram_tensor("target", [BL, T], i32, kind="ExternalInput")
    trans_d = nc.dram_tensor("transition", [L, L], fp32, kind="ExternalInput")
    logz_d = nc.dram_tensor("logz_row", [1, BL], fp32, kind="ExternalOutput")
    emitacc_d = nc.dram_tensor("emit_acc", [BL, 1], fp32, kind="ExternalOutput")
    transcol_d = nc.dram_tensor("trans_col", [L, 1], fp32, kind="ExternalOutput")

    with tile.TileContext(nc) as tc:
        with (
            tc.tile_pool(name="constp", bufs=1) as constp,
            tc.tile_pool(name="emtp", bufs=3) as emtp,
            tc.tile_pool(name="ettp", bufs=3) as ettp,
            tc.tile_pool(name="emgp", bufs=3) as emgp,
            tc.tile_pool(name="ohp", bufs=3) as ohp,
            tc.tile_pool(name="scrp", bufs=3) as scrp,
            tc.tile_pool(name="pp", bufs=8) as pp,
            tc.tile_pool(name="smallp", bufs=4) as smallp,
            tc.tile_pool(name="ps_scan", bufs=2, space="PSUM") as ps_scan,
            tc.tile_pool(name="ps_s", bufs=2, space="PSUM") as ps_s,
            tc.tile_pool(name="ps_r", bufs=1, space="PSUM") as ps_r,
            tc.tile_pool(name="ps_d", bufs=1, space="PSUM") as ps_d,
        ):
            # ---- constants -------------------------------------------------
            T_sb = constp.tile([L, L], fp32)
            nc.sync.dma_start(T_sb[:], trans_d[:])
            expT = constp.tile([L, L], bf16)
            nc.scalar.activation(expT[:], T_sb[:], AF.Exp)
            wstop = constp.tile([L, 1], bf16)
            nc.scalar.activation(wstop[:], T_sb[:, T_STOP : T_STOP + 1], AF.Exp)
            # row T_START of transition as a [L, 1] column, exponentiated
            r1col = constp.tile([L, 1], fp32)
            nc.sync.dma_start(
                r1col[:], trans_d[T_START : T_START + 1, :].rearrange("a b -> b a")
            )
            er1 = constp.tile([L, 1], fp32)
            nc.scalar.activation(er1[:], r1col[:], AF.Exp)
            ones_col = constp.tile([L, 1], bf16)
            nc.vector.memset(ones_col[:], 1.0)
            ones_row = constp.tile([1, L], fp32)
            nc.vector.memset(ones_row[:], 1.0)
            iota_i = constp.tile([2 * BL, L], i32)
            nc.gpsimd.iota(iota_i[:], pattern=[[1, L]], channel_multiplier=0)
            iota_f = constp.tile([2 * BL, L], bf16)
            nc.vector.tensor_copy(iota_f[:], iota_i[:])
            tgt_i = constp.tile([BL, T], i32)
            nc.sync.dma_start(tgt_i[:], tgt_d[:])
            # tgt2: rows 0..63 = target[b, t]; rows 64..127 = target[b, t+1]
            # (so a one-hot chunk of tgt2 holds both ends of every pair; the
            # final column of the shifted block is -1 = "matches nothing")
            tgt2 = constp.tile([2 * BL, T], bf16)
            nc.vector.tensor_copy(tgt2[:BL, :], tgt_i[:])
            nc.vector.tensor_copy(tgt2[BL:, : T - 1], tgt_i[:, 1:])
            nc.vector.memset(tgt2[BL:, T - 1 :], -1.0)
            Macc = constp.tile([1, BL], fp32)
            nc.vector.memset(Macc[:], 0.0)
            negc = constp.tile([L, 1], fp32)
            nc.vector.memset(negc[:], -C_SHIFT)
            # emit-score partial sums, one column per sub-op
            tmps = constp.tile([BL, NPART * nreps], fp32)

            D_ps = ps_d.tile([L, L], fp32)  # transition pair counts

            p_cur = [None] * NG
            sc = 0  # completed scan steps
            n_pair = 0  # D matmuls issued (of T//2, two pairs each)
            n_sub = 0  # emit-score sub-ops issued

            for ci in [c for _ in range(nreps) for c in range(NCHUNK)]:
                t0 = ci * TC

                # scan-side chunk: transposed emit [j, (t, b)]
                emT = emtp.tile([L, TC * BL], fp32, name="emT")
                nc.gpsimd.dma_start(emT[:], emitT_d[:, t0 : t0 + TC, :])
                etT = ettp.tile([L, TC * BL], bf16, name="etT")
                nc.scalar.activation(etT[:], emT[:], AF.Exp, bias=negc[:])

                # gold-side chunk: natural layout [b, (t, l)]
                em_g = emgp.tile([BL, TC * L], fp32, name="em_g")
                nc.gpsimd.dma_start(em_g[:], emit_d[:, t0 : t0 + TC, :])

                # one-hot tiles for the chunk (bf16: values 0..63 exact):
                # 128 partitions = (pair-end h, batch b), built in NSUB slices
                oh = ohp.tile([2 * BL, TC * L], bf16, name="oh")
                ohsub = []
                tcs = TC // NSUB
                for s in range(NSUB):
                    ohsub.append((s * tcs, (s + 1) * tcs))

                def oh_build(ta, tb):
                    nc.vector.tensor_tensor(
                        oh[:, ta * L : tb * L].rearrange("p (t l) -> p t l", l=L),
                        iota_f[:, None, :].to_broadcast([2 * BL, tb - ta, L]),
                        tgt2[:, t0 + ta : t0 + tb, None].to_broadcast(
                            [2 * BL, tb - ta, L]
                        ),
                        AOP.is_equal,
                    )

                # emit gold score sub-ranges (interleaved into the scan)
                lo = max(t0, 1)
                hi = min(t0 + TC, T - 1)
                scr = scrp.tile([BL, TC * L], bf16, name="scr")
                sub = []
                step8 = (hi - lo + NSUB - 1) // NSUB
                a = lo
                while a < hi:
                    b = min(a + step8, hi)
                    sub.append((a, b))
                    a = b

                def emit_sub(a, b):
                    # prod = em * oh on DVE; row-sum on ACT into tmps column
                    nonlocal n_sub
                    sl = slice((a - t0) * L, (b - t0) * L)
                    nc.vector.tensor_tensor(
                        scr[:, sl], em_g[:, sl], oh[:BL, sl], AOP.mult
                    )
                    nc.scalar.activation(
                        scr[:, sl],
                        scr[:, sl],
                        AF.Copy,
                        accum_out=tmps[:, n_sub : n_sub + 1],
                    )
                    n_sub += 1

                # transition-pair matmuls: lhsT/rhs span 128 partitions =
                # (onehot(t_k) , onehot(t_{k+1})), so one matmul accumulates
                # pairs (k, k+1) AND (k+1, k+2); stride 2 covers everything
                # including chunk boundaries (via the shifted block), and the
                # -1 sentinel blanks the nonexistent final pair.
                pairs = list(range(0, TC, 2))

                def pair_mm(k):
                    nonlocal n_pair
                    nc.tensor.matmul(
                        D_ps[:],
                        lhsT=oh[:, k * L : (k + 1) * L],
                        rhs=oh[:, (k + 1) * L : (k + 2) * L],
                        start=(n_pair == 0),
                        stop=(n_pair == nreps * T // 2 - 1),
                        skip_group_check=True,
                    )
                    n_pair += 1

                # ---- scan over this chunk's steps, interleaving gold work --
                n_built = 0  # one-hot slices built so far (cover t0 .. t0+8*n)

                def oh_step():
                    nonlocal n_built
                    oh_build(*ohsub.pop(0))
                    n_built += 1

                oh_step()  # slice 0 needed by the first (boundary) pair
                if ci == 0:
                    # p_1 = exp(emit_1 - c) * exp(transition[START, :])
                    for g in range(NG):
                        p1 = pp.tile([L, BG], bf16, name=f"p_g{g}")
                        nc.vector.tensor_scalar(
                            p1[:],
                            etT[:, BL + g * BG : BL + (g + 1) * BG],
                            er1[:],
                            None,
                            AOP.mult,
                        )
                        p_cur[g] = p1
                    sc = 1  # emit factors consumed so far (t=1)

                work = []
                for k in range(TC):
                    t = t0 + k
                    if 2 <= t <= T - 2:
                        work.append(("scan", k))
                    if pairs and k % 2 == 1:
                        work.append(("pair", pairs.pop(0)))

                nwork = max(len(work), 1)
                for wi, (kind, arg) in enumerate(work):
                    # keep one-hot slices built ~10 items ahead of the pair
                    # matmuls that read them; emit sub-ops trail behind
                    while ohsub and n_built * (TC // NSUB) <= wi + 12:
                        oh_step()
                    if sub and (NSUB - len(sub) + 1) * nwork // (NSUB + 1) <= wi:
                        emit_sub(*sub.pop(0))
                    if kind == "pair":
                        pair_mm(arg)
                        continue
                    k = arg
                    t = t0 + k
                    ps_g = []
                    for g in range(NG):
                        ps = ps_scan.tile([L, BG], fp32, name=f"ps_g{g}")
                        nc.tensor.matmul(
                            ps[:],
                            lhsT=expT[:],
                            rhs=p_cur[g][:],
                            start=True,
                            stop=True,
                            skip_group_check=True,
                        )
                        ps_g.append(ps)
                    for g in range(NG):
                        p_new = pp.tile([L, BG], bf16, name=f"p_g{g}")
                        nc.vector.tensor_tensor(
                            p_new[:],
                            ps_g[g][:],
                            etT[:, k * BL + g * BG : k * BL + (g + 1) * BG],
                            AOP.mult,
                        )
                        p_cur[g] = p_new
                    sc += 1
                    if sc % RENORM == 0:
                        S_ps = ps_s.tile([1, BL], fp32, name="S_ps")
                        for g in range(NG):
                            nc.tensor.matmul(
                                S_ps[:, g * BG : (g + 1) * BG],
                                lhsT=ones_col[:],
                                rhs=p_cur[g][:],
                                start=True,
                                stop=True,
                                skip_group_check=True,
                            )
                        lnS = smallp.tile([1, BL], fp32, name="lnS")
                        nc.scalar.activation(lnS[:], S_ps[:], AF.Ln)
                        nc.vector.tensor_tensor(Macc[:], Macc[:], lnS[:], AOP.add)
                        rS = smallp.tile([1, BL], fp32, name="rS")
                        nc.vector.reciprocal(rS[:], S_ps[:])
                        R_ps = ps_r.tile([L, BL], fp32, name="R_ps")
                        nc.tensor.matmul(
                            R_ps[:],
                            lhsT=ones_row[:],
                            rhs=rS[:],
                            start=True,
                            stop=True,
                            skip_group_check=True,
                        )
                        for g in range(NG):
                            p_rn = pp.tile([L, BG], bf16, name=f"p_g{g}")
                            nc.vector.tensor_tensor(
                                p_rn[:],
                                p_cur[g][:],
                                R_ps[:, g * BG : (g + 1) * BG],
                                AOP.mult,
                            )
                            p_cur[g] = p_rn

                # drain remaining sub-ops of the chunk
                while ohsub:
                    oh_build(*ohsub.pop(0))
                while sub:
                    emit_sub(*sub.pop(0))
                while pairs:
                    pair_mm(pairs.pop(0))

            assert n_pair == nreps * T // 2, n_pair
            assert nreps > 1 or sc == N_STEPS, sc

            # ---- close -----------------------------------------------------
            S_fin = ps_s.tile([1, BL], fp32, name="S_ps")
            for g in range(NG):
                nc.tensor.matmul(
                    S_fin[:, g * BG : (g + 1) * BG],
                    lhsT=wstop[:],
                    rhs=p_cur[g][:],
                    start=True,
                    stop=True,
                    skip_group_check=True,
                )
            lzf = smallp.tile([1, BL], fp32)
            nc.scalar.activation(lzf[:], S_fin[:], AF.Ln)
            lz_out = smallp.tile([1, BL], fp32)
            nc.vector.tensor_tensor(lz_out[:], lzf[:], Macc[:], AOP.add)
            nc.sync.dma_start(logz_d[:], lz_out[:])

            acc = constp.tile([BL, 1], fp32)
            nc.vector.reduce_sum(acc[:], tmps[:, :n_sub], axis=AX.X)
            nc.sync.dma_start(emitacc_d[:], acc[:])

            D_sb = constp.tile([L, L], fp32)
            nc.vector.tensor_copy(D_sb[:], D_ps[:])
            sc64 = constp.tile([L, L], fp32)
            nc.vector.tensor_tensor(sc64[:], D_sb[:], T_sb[:], AOP.mult)
            tcol = constp.tile([L, 1], fp32)
            nc.vector.reduce_sum(tcol[:], sc64[:], axis=AX.X)
            nc.sync.dma_start(transcol_d[:], tcol[:])

    _split_multi_waits(nc, mybir)
    return nc


def _get_nc():
    if "nc" not in _CACHE:
        _CACHE["nc"] = _build()
    return _CACHE["nc"]


def _make_in_maps(np_inputs):
    emit = np.ascontiguousarray(np_inputs["emit"], dtype=np.float32)
    tgt = np.ascontiguousarray(np_inputs["target"]).astype(np.int32)
    trans = np.ascontiguousarray(np_inputs["transition"], dtype=np.float32)
    assert emit.shape == (B, T, L) and tgt.shape == (B, T)
    in_maps = []
    for k in range(NCORES):
        esh = emit[k * BL : (k + 1) * BL]
        in_maps.append(
            {
                "emit": esh,
                "emitT": np.ascontiguousarray(esh.transpose(2, 1, 0)),
                "target": tgt[k * BL : (k + 1) * BL],
                "transition": trans,
            }
        )
    return in_maps


def kernel(emit, target, transition):
    from concourse import bass_utils

    nc = _get_nc()
    in_maps = _make_in_maps(
        {"emit": emit, "target": target, "transition": transition}
    )
    global LAST_EXEC_NS, LAST_RESULT
    res = bass_utils.run_bass_kernel_spmd(
        nc, in_maps, core_ids=list(range(NCORES)), trace=TRACE
    )
    LAST_EXEC_NS = res.exec_time_ns
    LAST_RESULT = res

    tot = 0.0
    for r in res.results:
        tot += float(r["logz_row"].astype(np.float64).sum())
        tot += BL * (T - 2) * C_SHIFT
        tot -= float(r["emit_acc"].astype(np.float64).sum())
        tot -= float(r["trans_col"].astype(np.float64).sum())
    return np.float32(tot)

